# revision 4
# baseline (speedup 1.0000x reference)
"""Trainium2 Bass kernel for nn_DAMWrapper (symmetric-Toeplitz attention-distance masks).

Math: per head h, keep-prob m[h,d] = softmax((alphas + gumbel)/tau, axis=-1)[...,0]
     = sigmoid((a0 - a1) - log(e0+eps) + log(e1+eps)), d in [0,N).
Outputs (both [H, N, N] f32):  masks[h,i,j] = m[h,|i-j|]
                               mask_normalize = (1 - masks) * -10000.

Strategy: the big tensors are never computed elementwise. Per head we build an
SBUF tensor S[p,k] = v[k-1-p] where v is the length-(2N-1) reflection of m
(v[x] = m[|x-(N-1)|]). Every 128-row output tile is then a plain sliding-window
slice S[:, o_t:o_t+N] stored straight to HBM, so the kernel is pure DMA at the
HBM-write roofline. S itself is built with log-doubling partition-shifted
SBUF->SBUF copies from row 0.

Sharding: H=16 heads split over 8 NeuronCores (2 heads each), SPMD.
"""

import numpy as np

import jax

import concourse.bacc as bacc
import concourse.bass as bass
import concourse.mybir as mybir
import concourse.tile as tile
from concourse.bass_utils import run_bass_kernel_spmd

# Persistent XLA compile cache: repeat kernel() calls (same HLO, which embeds
# the BIR) skip the minutes-long neuronx-cc recompile.
try:
    jax.config.update("jax_compilation_cache_dir", "/tmp/jax_comp_cache")
    jax.config.update("jax_persistent_cache_min_compile_time_secs", 0.0)
    jax.config.update("jax_persistent_cache_min_entry_size_bytes", 0)
except Exception:
    pass

AF = mybir.ActivationFunctionType
dt = mybir.dt

H = 16
N = 2048
P = 128
N_CORES = 8
H_LOC = H // N_CORES  # heads per core
Q = N // P            # free elems per partition for the m layout
W = 2 * N             # S width
NT = N // P           # 128-row tiles per head
EPS = 1e-5

_CACHE = {}


def _build_bass(repeat=1, setup_repeat=1, v2=False, v2_gpsimd=None, v2_stage1=None):
    """repeat/setup_repeat>1 re-issue the fill DMAs / S-build (benchmarking
    aids: device-side time = d(wall)/d(repeat); grading always uses 1/1).
    v2: memset/S_w on gpsimd (off the DVE critical path) + stage-1 forward
    halves copied straight from m_t so they overlap the DVE reversal."""
    import os
    _B = int(os.environ.get('DAM_B', '16'))
    v2_gpsimd = v2 if v2_gpsimd is None else v2_gpsimd
    v2_stage1 = v2 if v2_stage1 is None else v2_stage1
    nc = bacc.Bacc("TRN2", target_bir_lowering=False, debug=False)
    alphas = nc.dram_tensor(
        "init_alphas", [H_LOC, N, 2], dt.float32, kind="ExternalInput"
    )
    noise = nc.dram_tensor(
        "exp_noise", [H_LOC, N, 2], dt.float32, kind="ExternalInput"
    )
    maskn = nc.dram_tensor(
        "mask_normalize", [H_LOC, N, N], dt.float32, kind="ExternalOutput"
    )
    masks = nc.dram_tensor("masks", [H_LOC, N, N], dt.float32, kind="ExternalOutput")

    with tile.TileContext(nc) as tc:
        with tc.tile_pool(name="pool", bufs=1) as pool:
            a_t = pool.tile([P, H_LOC, Q, 2], dt.float32)
            n_t = pool.tile([P, H_LOC, Q, 2], dt.float32)
            nc.sync.dma_start(out=a_t[:], in_=alphas.rearrange("h (p q) e -> p h q e", p=P))
            nc.sync.dma_start(out=n_t[:], in_=noise.rearrange("h (p q) e -> p h q e", p=P))

            eps_t = pool.tile([P, 1], dt.float32)
            nc.vector.memset(eps_t[:], EPS)

            # logits = alphas - log(noise + EPS); m = sigmoid(l0 - l1)
            # (computed per head when DAM_MSPLIT=1 so head 0's S-build can
            # begin before head 1's logits finish)
            _msplit = os.environ.get('DAM_MSPLIT', '0') == '1'
            lg = pool.tile([P, H_LOC, Q, 2], dt.float32)
            m_t = pool.tile([P, H_LOC, Q], dt.float32)
            _hs = [slice(h, h + 1) for h in range(H_LOC)] if _msplit else [slice(None)]
            for _h in _hs:
                nc.scalar.activation(out=lg[:, _h], in_=n_t[:, _h], func=AF.Ln, bias=eps_t[:], scale=1.0)
                nc.vector.tensor_sub(lg[:, _h], a_t[:, _h], lg[:, _h])
                nc.vector.tensor_sub(m_t[:, _h], lg[:, _h, :, 0], lg[:, _h, :, 1])
                nc.scalar.activation(out=m_t[:, _h], in_=m_t[:, _h], func=AF.Sigmoid)

            B = _B  # stage-1 seeds rows 1..B-1, stage-2 copies B-row blocks
            S_vs, S_ws, engs = [], [], []
            for h in range(H_LOC):
                # head h's DMAs ride their own HWDGE ring (SP / ACT) so the
                # two heads' dependency chains never stall each other
                eng = nc.sync if h % 2 == 0 else nc.scalar
                engs.append(eng)
                WPAD = W + B  # stage-1 fwd rows write up to col W+B-1; fills read < W
                S_v = pool.tile([P, WPAD], dt.float32, name=f"S_v{h}", tag=f"S_v{h}")
                S_w = pool.tile([P, WPAD], dt.float32, name=f"S_w{h}", tag=f"S_w{h}")
                S_vs.append(S_v)
                S_ws.append(S_w)
                for _ in range(setup_repeat):
                    # zero the (never-read) garbage triangle k < p+1 so no junk
                    # values flow through the block copies
                    (nc.gpsimd if v2_gpsimd else nc.vector).memset(S_v[:, 0:P], 0.0)
                    # row 0 = v shifted by 1: S_v[0,k] = m[|k-N|]
                    # forward half S_v[0, N+n] = m[n] (128p -> 1p gather DMA)
                    eng.dma_start(out=S_v[0:1, N : 2 * N], in_=m_t[:, h, :])
                    if v2_stage1:
                        # stage-1 forward halves straight from m_t: independent
                        # of the DVE reversal, so they overlap it
                        for d in range(1, B):
                            eng.dma_start(
                                out=S_v[d : d + 1, N + d : 2 * N + d],
                                in_=m_t[:, h, :],
                            )
                    # mirrored half via an in-partition reversed DVE copy:
                    # S_v[0, k] = S_v[0, 2W-1-k] for k in [1, N-1]
                    pstep = S_v.ap[0][0]
                    rev_src = bass.AP(
                        S_v.tensor, S_v.offset + W - 1, [[pstep, 1], [-1, N - 1]]
                    )
                    nc.vector.tensor_copy(S_v[0:1, 1:N], rev_src)
                    # stage 1: remaining (mirrored-side) spans of rows 1..B-1
                    for d in range(1, B):
                        if v2_stage1:
                            eng.dma_start(
                                out=S_v[d : d + 1, d : N + d], in_=S_v[0:1, 0:N]
                            )
                        else:
                            eng.dma_start(
                                out=S_v[d : d + 1, d:W], in_=S_v[0:1, 0 : W - d]
                            )
                    # stage 2: B-row blocks, all independent reads of rows 0..B-1
                    for b in range(1, P // B):
                        d = B * b
                        eng.dma_start(
                            out=S_v[d : d + B, d:W], in_=S_v[0:B, 0 : W - d]
                        )
                    # S_w = (S_v - 1) * 1e4 — bit-identical to (1 - S_v) * -1e4
                    (nc.gpsimd if v2_gpsimd else nc.vector).tensor_scalar(
                        S_w[:, 0:W], S_v[:, 0:W], 1.0, 1.0e4,
                        mybir.AluOpType.subtract, mybir.AluOpType.mult,
                    )
            # sliding-window Toeplitz fills (pure DMA, HBM-write bound).
            # masks fills first: the maskn fills wait on S_w, and a stalled
            # DMA at the head of a ring blocks everything behind it.
            _fuse = os.environ.get('DAM_FUSE', '0') == '1'

            def _fused_src(S):
                # one 16 MiB DMA per (head, output): src AP iterates
                # (p, t, j) -> S[p, N - P*t + j]; partition dim first so the
                # matching dest AP [[N,P],[P*N,NT],[1,N]] stays 3-dim and
                # non-collapsible. The t dim slides backward (-P).
                pstep = S.ap[0][0]
                return bass.AP(S.tensor, S.offset + N, [[pstep, P], [-P, NT], [1, N]])

            def _fused_dst(out_dram, h):
                return out_dram.rearrange("h (t p) n -> h p t n", p=P)[h]

            def _fused_src_part(S, ta, tb):
                pstep = S.ap[0][0]
                return bass.AP(
                    S.tensor, S.offset + N - P * ta, [[pstep, P], [-P, tb - ta], [1, N]]
                )

            def _fused_dst_part(out_dram, h, ta, tb):
                ap = out_dram[h, :, :]
                return bass.AP(
                    ap.tensor, ap.offset + ta * P * N, [[N, P], [P * N, tb - ta], [1, N]]
                )

            # DAM_GT=g: in fused mode, tiles [0,g) of each maskn stream go to
            # the gpsimd SWDGE queue (3rd DMA queue); 0 disables.
            _gt = int(os.environ.get('DAM_GT', '0'))
            for _ in range(repeat):
                if _fuse:
                    for h in range(H_LOC):
                        engs[h].dma_start(
                            out=_fused_dst(masks, h), in_=_fused_src(S_vs[h])
                        )
                    for h in range(H_LOC):
                        if _gt > 0:
                            nc.gpsimd.dma_start(
                                out=_fused_dst_part(maskn, h, 0, _gt),
                                in_=_fused_src_part(S_ws[h], 0, _gt),
                            )
                            engs[h].dma_start(
                                out=_fused_dst_part(maskn, h, _gt, NT),
                                in_=_fused_src_part(S_ws[h], _gt, NT),
                            )
                        else:
                            engs[h].dma_start(
                                out=_fused_dst(maskn, h), in_=_fused_src(S_ws[h])
                            )
                else:
                    for h in range(H_LOC):
                        for t in range(NT):
                            o_t = N - P * t
                            engs[h].dma_start(
                                out=masks[h, P * t : P * (t + 1), :],
                                in_=S_vs[h][:, o_t : o_t + N],
                            )
                    for h in range(H_LOC):
                        for t in range(NT):
                            o_t = N - P * t
                            engs[h].dma_start(
                                out=maskn[h, P * t : P * (t + 1), :],
                                in_=S_ws[h][:, o_t : o_t + N],
                            )
    nc.compile()
    return nc


def _get_nc():
    if "nc" not in _CACHE:
        _CACHE["nc"] = _build_bass()
    return _CACHE["nc"]


def kernel(init_alphas, exp_noise, _run_kwargs=None):
    init_alphas = np.ascontiguousarray(init_alphas, dtype=np.float32)
    exp_noise = np.ascontiguousarray(exp_noise, dtype=np.float32)
    nc = _get_nc()
    in_maps = [
        {
            "init_alphas": np.ascontiguousarray(
                init_alphas[c * H_LOC : (c + 1) * H_LOC]
            ),
            "exp_noise": np.ascontiguousarray(exp_noise[c * H_LOC : (c + 1) * H_LOC]),
        }
        for c in range(N_CORES)
    ]
    res = run_bass_kernel_spmd(
        nc, in_maps, core_ids=list(range(N_CORES)), **(_run_kwargs or {})
    )
    maskn = np.concatenate([r["mask_normalize"] for r in res.results], axis=0)
    masks = np.concatenate([r["masks"] for r in res.results], axis=0)
    if _run_kwargs:
        _CACHE["last_results"] = res
    return maskn, masks



# revision 7
# speedup vs baseline: 1.0416x; 1.0416x over previous
"""Trainium2 Bass kernel for nn_DAMWrapper (symmetric-Toeplitz attention-distance masks).

Math: per head h, keep-prob m[h,d] = softmax((alphas + gumbel)/tau, axis=-1)[...,0]
     = sigmoid((a0 - a1) - log(e0+eps) + log(e1+eps)), d in [0,N).
Outputs (both [H, N, N] f32):  masks[h,i,j] = m[h,|i-j|]
                               mask_normalize = (1 - masks) * -10000.

Strategy: the big tensors are never computed elementwise. Per head we build an
SBUF tensor S[p,k] = v[k-1-p] where v is the length-(2N-1) reflection of m
(v[x] = m[|x-(N-1)|]). Every 128-row output tile is then a plain sliding-window
slice S[:, o_t:o_t+N] stored straight to HBM, so the kernel is pure DMA at the
HBM-write roofline. S itself is built with log-doubling partition-shifted
SBUF->SBUF copies from row 0.

Sharding: H=16 heads split over 8 NeuronCores (2 heads each), SPMD.
"""

import numpy as np

import jax

import concourse.bacc as bacc
import concourse.bass as bass
import concourse.mybir as mybir
import concourse.tile as tile
from concourse.bass_utils import run_bass_kernel_spmd

# Persistent XLA compile cache: repeat kernel() calls (same HLO, which embeds
# the BIR) skip the minutes-long neuronx-cc recompile.
try:
    jax.config.update("jax_compilation_cache_dir", "/tmp/jax_comp_cache")
    jax.config.update("jax_persistent_cache_min_compile_time_secs", 0.0)
    jax.config.update("jax_persistent_cache_min_entry_size_bytes", 0)
except Exception:
    pass

AF = mybir.ActivationFunctionType
dt = mybir.dt

H = 16
N = 2048
P = 128
N_CORES = 8
H_LOC = H // N_CORES  # heads per core
Q = N // P            # free elems per partition for the m layout
W = 2 * N             # S width
NT = N // P           # 128-row tiles per head
EPS = 1e-5

_CACHE = {}


def _build_bass(repeat=1, setup_repeat=1, v2=False, v2_gpsimd=None, v2_stage1=None):
    """repeat/setup_repeat>1 re-issue the fill DMAs / S-build (benchmarking
    aids: device-side time = d(wall)/d(repeat); grading always uses 1/1).
    v2: memset/S_w on gpsimd (off the DVE critical path) + stage-1 forward
    halves copied straight from m_t so they overlap the DVE reversal."""
    import os
    _B = int(os.environ.get('DAM_B', '16'))
    v2_gpsimd = v2 if v2_gpsimd is None else v2_gpsimd
    v2_stage1 = v2 if v2_stage1 is None else v2_stage1
    nc = bacc.Bacc("TRN2", target_bir_lowering=False, debug=False)
    alphas = nc.dram_tensor(
        "init_alphas", [H_LOC, N, 2], dt.float32, kind="ExternalInput"
    )
    noise = nc.dram_tensor(
        "exp_noise", [H_LOC, N, 2], dt.float32, kind="ExternalInput"
    )
    maskn = nc.dram_tensor(
        "mask_normalize", [H_LOC, N, N], dt.float32, kind="ExternalOutput"
    )
    masks = nc.dram_tensor("masks", [H_LOC, N, N], dt.float32, kind="ExternalOutput")

    with tile.TileContext(nc) as tc:
        with tc.tile_pool(name="pool", bufs=1) as pool:
            a_t = pool.tile([P, H_LOC, Q, 2], dt.float32)
            n_t = pool.tile([P, H_LOC, Q, 2], dt.float32)
            nc.sync.dma_start(out=a_t[:], in_=alphas.rearrange("h (p q) e -> p h q e", p=P))
            nc.sync.dma_start(out=n_t[:], in_=noise.rearrange("h (p q) e -> p h q e", p=P))

            eps_t = pool.tile([P, 1], dt.float32)
            nc.vector.memset(eps_t[:], EPS)

            # logits = alphas - log(noise + EPS); m = sigmoid(l0 - l1)
            # (computed per head when DAM_MSPLIT=1 so head 0's S-build can
            # begin before head 1's logits finish)
            _msplit = os.environ.get('DAM_MSPLIT', '0') == '1'
            lg = pool.tile([P, H_LOC, Q, 2], dt.float32)
            m_t = pool.tile([P, H_LOC, Q], dt.float32)
            _hs = [slice(h, h + 1) for h in range(H_LOC)] if _msplit else [slice(None)]
            for _h in _hs:
                nc.scalar.activation(out=lg[:, _h], in_=n_t[:, _h], func=AF.Ln, bias=eps_t[:], scale=1.0)
                nc.vector.tensor_sub(lg[:, _h], a_t[:, _h], lg[:, _h])
                nc.vector.tensor_sub(m_t[:, _h], lg[:, _h, :, 0], lg[:, _h, :, 1])
                nc.scalar.activation(out=m_t[:, _h], in_=m_t[:, _h], func=AF.Sigmoid)

            B = _B  # stage-1 seeds rows 1..B-1, stage-2 copies B-row blocks
            S_vs, S_ws, engs = [], [], []
            for h in range(H_LOC):
                # head h's DMAs ride their own HWDGE ring (SP / ACT) so the
                # two heads' dependency chains never stall each other
                eng = nc.sync if h % 2 == 0 else nc.scalar
                engs.append(eng)
                WPAD = W + B  # stage-1 fwd rows write up to col W+B-1; fills read < W
                S_v = pool.tile([P, WPAD], dt.float32, name=f"S_v{h}", tag=f"S_v{h}")
                S_w = pool.tile([P, WPAD], dt.float32, name=f"S_w{h}", tag=f"S_w{h}")
                S_vs.append(S_v)
                S_ws.append(S_w)
                for _ in range(setup_repeat):
                    # zero the (never-read) garbage triangle k < p+1 so no junk
                    # values flow through the block copies
                    (nc.gpsimd if v2_gpsimd else nc.vector).memset(S_v[:, 0:P], 0.0)
                    # row 0 = v shifted by 1: S_v[0,k] = m[|k-N|]
                    # forward half S_v[0, N+n] = m[n] (128p -> 1p gather DMA)
                    eng.dma_start(out=S_v[0:1, N : 2 * N], in_=m_t[:, h, :])
                    if v2_stage1:
                        # stage-1 forward halves straight from m_t: independent
                        # of the DVE reversal, so they overlap it
                        for d in range(1, B):
                            eng.dma_start(
                                out=S_v[d : d + 1, N + d : 2 * N + d],
                                in_=m_t[:, h, :],
                            )
                    # mirrored half via an in-partition reversed DVE copy:
                    # S_v[0, k] = S_v[0, 2W-1-k] for k in [1, N-1]
                    pstep = S_v.ap[0][0]
                    rev_src = bass.AP(
                        S_v.tensor, S_v.offset + W - 1, [[pstep, 1], [-1, N - 1]]
                    )
                    nc.vector.tensor_copy(S_v[0:1, 1:N], rev_src)
                    # stage 1: remaining (mirrored-side) spans of rows 1..B-1
                    for d in range(1, B):
                        if v2_stage1:
                            eng.dma_start(
                                out=S_v[d : d + 1, d : N + d], in_=S_v[0:1, 0:N]
                            )
                        else:
                            eng.dma_start(
                                out=S_v[d : d + 1, d:W], in_=S_v[0:1, 0 : W - d]
                            )
                    # stage 2: B-row blocks, all independent reads of rows 0..B-1
                    for b in range(1, P // B):
                        d = B * b
                        eng.dma_start(
                            out=S_v[d : d + B, d:W], in_=S_v[0:B, 0 : W - d]
                        )
                    # S_w = (S_v - 1) * 1e4 — bit-identical to (1 - S_v) * -1e4
                    (nc.gpsimd if v2_gpsimd else nc.vector).tensor_scalar(
                        S_w[:, 0:W], S_v[:, 0:W], 1.0, 1.0e4,
                        mybir.AluOpType.subtract, mybir.AluOpType.mult,
                    )
            # sliding-window Toeplitz fills (pure DMA, HBM-write bound).
            # masks fills first: the maskn fills wait on S_w, and a stalled
            # DMA at the head of a ring blocks everything behind it.
            _fuse = os.environ.get('DAM_FUSE', '0') == '1'

            def _fused_src(S):
                # one 16 MiB DMA per (head, output): src AP iterates
                # (p, t, j) -> S[p, N - P*t + j]; partition dim first so the
                # matching dest AP [[N,P],[P*N,NT],[1,N]] stays 3-dim and
                # non-collapsible. The t dim slides backward (-P).
                pstep = S.ap[0][0]
                return bass.AP(S.tensor, S.offset + N, [[pstep, P], [-P, NT], [1, N]])

            def _fused_dst(out_dram, h):
                return out_dram.rearrange("h (t p) n -> h p t n", p=P)[h]

            def _fused_src_tmaj(S):
                # t-major order, t iterated reversed (u = NT-1-t) so the
                # negative stride lands on the DRAM dest: src (u, p, j) ->
                # S[p, 128 + P*u + j]; dest row-block 15-u.
                pstep = S.ap[0][0]
                return bass.AP(
                    S.tensor, S.offset + N - P * (NT - 1), [[P, NT], [pstep, P], [1, N]]
                )

            def _fused_dst_tmaj(out_dram, h):
                ap = out_dram[h, :, :]
                return bass.AP(
                    ap.tensor,
                    ap.offset + (NT - 1) * P * N,
                    [[-P * N, NT], [N, P], [1, N]],
                )

            def _fused_src_part(S, ta, tb):
                pstep = S.ap[0][0]
                return bass.AP(
                    S.tensor, S.offset + N - P * ta, [[pstep, P], [-P, tb - ta], [1, N]]
                )

            def _fused_dst_part(out_dram, h, ta, tb):
                ap = out_dram[h, :, :]
                return bass.AP(
                    ap.tensor, ap.offset + ta * P * N, [[N, P], [P * N, tb - ta], [1, N]]
                )

            # DAM_GT=g: in fused mode, tiles [0,g) of each maskn stream go to
            # the gpsimd SWDGE queue (3rd DMA queue); 0 disables.
            # DAM_RING1=1: all fills on the sync ring (single HWDGE queue).
            # DAM_X=1: cross-pair streams (SP: masks0+maskn1, ACT: masks1+maskn0)
            # so concurrent ring writes land in far-apart HBM regions.
            _gt = int(os.environ.get('DAM_GT', '0'))
            _ring1 = os.environ.get('DAM_RING1', '0') == '1'
            _cross = os.environ.get('DAM_X', '0') == '1'
            _tmaj = os.environ.get('DAM_TMAJ', '0') == '1'
            if _tmaj:
                _fused_src, _fused_dst = _fused_src_tmaj, _fused_dst_tmaj
            for _ in range(repeat):
                if _fuse:
                    for h in range(H_LOC):
                        eng = nc.sync if _ring1 else engs[h]
                        eng.dma_start(
                            out=_fused_dst(masks, h), in_=_fused_src(S_vs[h])
                        )
                    for h in range(H_LOC):
                        if _ring1:
                            eng = nc.sync
                        elif _cross:
                            eng = engs[1 - h]
                        else:
                            eng = engs[h]
                        if _gt > 0:
                            nc.gpsimd.dma_start(
                                out=_fused_dst_part(maskn, h, 0, _gt),
                                in_=_fused_src_part(S_ws[h], 0, _gt),
                            )
                            eng.dma_start(
                                out=_fused_dst_part(maskn, h, _gt, NT),
                                in_=_fused_src_part(S_ws[h], _gt, NT),
                            )
                        else:
                            eng.dma_start(
                                out=_fused_dst(maskn, h), in_=_fused_src(S_ws[h])
                            )
                else:
                    for h in range(H_LOC):
                        for t in range(NT):
                            o_t = N - P * t
                            engs[h].dma_start(
                                out=masks[h, P * t : P * (t + 1), :],
                                in_=S_vs[h][:, o_t : o_t + N],
                            )
                    for h in range(H_LOC):
                        for t in range(NT):
                            o_t = N - P * t
                            engs[h].dma_start(
                                out=maskn[h, P * t : P * (t + 1), :],
                                in_=S_ws[h][:, o_t : o_t + N],
                            )
    nc.compile()
    return nc


def _get_nc():
    if "nc" not in _CACHE:
        _CACHE["nc"] = _build_bass()
    return _CACHE["nc"]


def kernel(init_alphas, exp_noise, _run_kwargs=None):
    init_alphas = np.ascontiguousarray(init_alphas, dtype=np.float32)
    exp_noise = np.ascontiguousarray(exp_noise, dtype=np.float32)
    nc = _get_nc()
    in_maps = [
        {
            "init_alphas": np.ascontiguousarray(
                init_alphas[c * H_LOC : (c + 1) * H_LOC]
            ),
            "exp_noise": np.ascontiguousarray(exp_noise[c * H_LOC : (c + 1) * H_LOC]),
        }
        for c in range(N_CORES)
    ]
    res = run_bass_kernel_spmd(
        nc, in_maps, core_ids=list(range(N_CORES)), **(_run_kwargs or {})
    )
    maskn = np.concatenate([r["mask_normalize"] for r in res.results], axis=0)
    masks = np.concatenate([r["masks"] for r in res.results], axis=0)
    if _run_kwargs:
        _CACHE["last_results"] = res
    return maskn, masks

